# revision 26
# baseline (speedup 1.0000x reference)
"""MoE-attention (nn_MoEAttention) Trainium2 kernel.

Strategy: data-parallel over tokens across 8 NeuronCores. T = B*N = 4096
tokens; each core owns 512 consecutive query tokens (cores 0-3 -> batch 0,
cores 4-7 -> batch 1), so attention needs no cross-core traffic: every core
gets its batch's full [N, C] activations (transposed on host) to build the
shared K/V.

On-device per core (all FLOPs on device):
  - gating: fp32 logits -> softmax -> top-12 mask (max8/match_replace x2)
    -> dense gates g[t, e] (zero for unselected experts)
  - aux-loss partials: p_sum[E], freqs[E], sum(lse^2) via ones-vector
    matmul partition reductions (combined across cores on the host)
  - per-expert q projections (bf16, expert pairs packed on partitions)
  - attention per expert in transposed layout: S^T[keys, tok] = K^T.T@Q^T,
    P = exp(S^T) (no max subtraction needed: |scores| <= ~2), O^T = V_aug^T@P
    where V_aug carries a ones column so the softmax denominator falls out of
    the same matmul; normalization and the gate are folded into one
    per-token scale applied to O^T.
  - gated combine through W_out with expert pairs packed on the contraction
    axis (full 128-deep matmuls), accumulated in PSUM across experts.

The non-selected experts are computed too (dense) but contribute exactly 0
through g[t, e] = 0 - this avoids any on-device gather/scatter.
"""

import os
from contextlib import ExitStack

import numpy as np

import concourse.bass as bass
import concourse.mybir as mybir
import concourse.tile as tile
from concourse import bacc
from concourse.bass_utils import run_bass_kernel_spmd
from concourse.masks import make_identity

FP32 = mybir.dt.float32
BF16 = mybir.dt.bfloat16
ALU = mybir.AluOpType
AFT = mybir.ActivationFunctionType
AXX = mybir.AxisListType.X

B, N, C, E, D = 2, 2048, 768, 24, 64
H = 12                      # top-k experts per token
T = B * N                   # 4096 tokens
NCORES = 8
TQ = T // NCORES            # 512 query tokens per core
KC = C // 128               # 6 contraction chunks over C
NKC = N // 128              # 16 key chunks
MT = TQ // 128              # 4 output m-tiles per core
NPAIR = E // 2              # 12 expert pairs
SCALE = D ** -0.5

SWITCHLOSS = 0.1
ZLOSS = 0.001


def _emit(tc, aps):
    nc = tc.nc
    (xT, xqT, w_gate, W_in, b_in_pairs, W_out, b_out, W_kv, b_k, b_v,
     y_o, p_sum_o, freqs_o, zsq_o) = aps

    with ExitStack() as ctx:
        persist = ctx.enter_context(tc.tile_pool(name="persist", bufs=1))
        small = ctx.enter_context(tc.tile_pool(name="small", bufs=4))
        rda_pool = ctx.enter_context(tc.tile_pool(name="rda", bufs=2))
        pt_pool = ctx.enter_context(tc.tile_pool(name="pt", bufs=2))
        a64_pool = ctx.enter_context(tc.tile_pool(name="a64", bufs=2))
        y_pool = ctx.enter_context(tc.tile_pool(name="ysb", bufs=2))

        # ---- persistent SBUF tensors ----
        xbf = persist.tile([128, KC * N], BF16, tag="xbf")
        xqf = persist.tile([128, KC * TQ], FP32, tag="xqf")
        xqbf = persist.tile([128, KC * TQ], BF16, tag="xqbf")
        winbf = persist.tile([128, NPAIR * KC * 128], BF16, tag="winbf")
        woutbf = persist.tile([128, NPAIR * C], BF16, tag="woutbf")
        wkvbf = persist.tile([128, KC * 128], BF16, tag="wkvbf")
        wk2 = persist.tile([128, KC * 128], BF16, tag="wk2")
        wg = persist.tile([128, KC * E], FP32, tag="wg")
        # K^T duplicated on both partition halves so either Q half (even /
        # odd expert of a packed pair) can contract against it.
        ktbf = persist.tile([128, N], BF16, tag="ktbf")
        vauge = persist.tile([128, NKC * 65], BF16, tag="vauge")
        qt2 = persist.tile([128, NPAIR * TQ], BF16, tag="qt2")
        ot2 = persist.tile([128, NPAIR * TQ], BF16, tag="ot2")
        gT = persist.tile([E, TQ], FP32, tag="gT")
        gTbf = persist.tile([E, TQ], BF16, tag="gTbf")
        boutbf = persist.tile([E, C], BF16, tag="boutbf")
        binp = persist.tile([128, NPAIR], FP32, tag="binp")
        bksb = persist.tile([128, 1], FP32, tag="bksb")
        bvbf = persist.tile([1, D], BF16, tag="bvbf")
        ident = persist.tile([128, 128], FP32, tag="ident")
        ones1 = persist.tile([128, 1], FP32, tag="ones1")
        ones64 = persist.tile([128, 64], BF16, tag="ones64")
        onesr = persist.tile([1, 128], BF16, tag="onesr")

        make_identity(nc, ident[:])
        nc.vector.memset(ones1[:], 1.0)
        nc.vector.memset(ones64[:], 1.0)
        nc.vector.memset(onesr[:], 1.0)

        # ---- DMA loads (gpsimd DMAs cast fp32 -> bf16 in flight) ----
        for kc in range(KC):
            cs = slice(kc * 128, (kc + 1) * 128)
            nc.gpsimd.dma_start(out=xbf[:, kc * N:(kc + 1) * N], in_=xT[cs, :])
            nc.sync.dma_start(out=xqf[:, kc * TQ:(kc + 1) * TQ], in_=xqT[cs, :])
            nc.gpsimd.dma_start(out=xqbf[:, kc * TQ:(kc + 1) * TQ], in_=xqT[cs, :])
            nc.gpsimd.dma_start(out=wkvbf[:, kc * 128:(kc + 1) * 128], in_=W_kv[cs, :])
            nc.gpsimd.dma_start(out=wk2[:, kc * 128:kc * 128 + 64], in_=W_kv[cs, 0:64])
            nc.gpsimd.dma_start(out=wk2[:, kc * 128 + 64:(kc + 1) * 128], in_=W_kv[cs, 0:64])
            nc.sync.dma_start(out=wg[:, kc * E:(kc + 1) * E], in_=w_gate[cs, :])
        for p in range(NPAIR):
            for kc in range(KC):
                col = (p * KC + kc) * 128
                cs = slice(kc * 128, (kc + 1) * 128)
                nc.gpsimd.dma_start(out=winbf[:, col:col + 64], in_=W_in[2 * p, cs, :])
                nc.gpsimd.dma_start(out=winbf[:, col + 64:col + 128], in_=W_in[2 * p + 1, cs, :])
            nc.gpsimd.dma_start(out=woutbf[0:64, p * C:(p + 1) * C], in_=W_out[2 * p])
            nc.gpsimd.dma_start(out=woutbf[64:128, p * C:(p + 1) * C], in_=W_out[2 * p + 1])
        nc.sync.dma_start(out=binp[:], in_=b_in_pairs[:])
        nc.sync.dma_start(out=bksb[:], in_=b_k[:])
        nc.gpsimd.dma_start(out=bvbf[:], in_=b_v[:])
        nc.gpsimd.dma_start(out=boutbf[:], in_=b_out[:])

        # ---- stage 1: K/V, gating, per-expert q (pairs packed on M) ----
        with tc.tile_pool(name="ps1", bufs=1, space="PSUM") as ps1, \
             tc.tile_pool(name="ps1b", bufs=2, space="PSUM") as ps1b:

            # K^T [128, N] (D-major, duplicated halves), scale/bias folded in
            for nb in range(N // 512):
                ktp = ps1b.tile([128, 512], FP32, tag="kt")
                for kc in range(KC):
                    nc.tensor.matmul(
                        ktp[:],
                        wk2[:, kc * 128:(kc + 1) * 128],
                        xbf[:, kc * N + nb * 512: kc * N + (nb + 1) * 512],
                        start=(kc == 0), stop=(kc == KC - 1))
                nc.vector.tensor_scalar(
                    out=ktbf[:, nb * 512:(nb + 1) * 512], in0=ktp[:],
                    scalar1=bksb[:], scalar2=SCALE, op0=ALU.add, op1=ALU.mult)

            # V [keys, 64] -> V_aug variants (ones column for the denominator)
            for kk in range(NKC):
                vps = ps1.tile([128, 64], FP32, tag="vps")
                for kc in range(KC):
                    nc.tensor.matmul(
                        vps[:],
                        xbf[:, kc * N + kk * 128: kc * N + (kk + 1) * 128],
                        wkvbf[:, kc * 128 + 64: kc * 128 + 128],
                        start=(kc == 0), stop=False)
                nc.tensor.matmul(vps[:], onesr[:], bvbf[:], start=False, stop=True)
                nc.vector.tensor_copy(out=vauge[:, kk * 65: kk * 65 + 64], in_=vps[:])
                nc.vector.memset(vauge[:, kk * 65 + 64: kk * 65 + 65], 1.0)

            # gating (exact fp32), top-12 mask, gates, loss partials
            stats = ps1.tile([1, 2 * E + 1], FP32, tag="stats")
            gtp = ps1.tile([E, TQ], FP32, tag="gtp")
            for m in range(MT):
                lg = ps1.tile([128, E], FP32, tag="lg")
                for kc in range(KC):
                    nc.tensor.matmul(
                        lg[:],
                        xqf[:, kc * TQ + m * 128: kc * TQ + (m + 1) * 128],
                        wg[:, kc * E:(kc + 1) * E],
                        start=(kc == 0), stop=(kc == KC - 1))
                mx = small.tile([128, 1], FP32, tag="mx")
                nc.vector.reduce_max(out=mx[:], in_=lg[:], axis=AXX)
                negmx = small.tile([128, 1], FP32, tag="negmx")
                nc.vector.tensor_scalar_mul(negmx[:], mx[:], -1.0)
                se = small.tile([128, 1], FP32, tag="se")
                # stin packs [probs | mask01 | lse^2] so the three loss
                # partials ride one PE accumulation group.
                stin = small.tile([128, 2 * E + 1], FP32, tag="stin")
                pr = stin[:, 0:E]
                nc.scalar.activation(pr, lg[:], AFT.Exp, bias=negmx[:], scale=1.0,
                                     accum_out=se[:])
                rs = small.tile([128, 1], FP32, tag="rs")
                nc.vector.reciprocal(rs[:], se[:])
                nc.vector.tensor_scalar_mul(pr, pr, rs[:])
                # z-loss partial: (log(sum_exp) + max)^2
                lnse = small.tile([128, 1], FP32, tag="lnse")
                nc.scalar.activation(lnse[:], se[:], AFT.Ln)
                lse = small.tile([128, 1], FP32, tag="lse")
                nc.vector.tensor_add(lse[:], lnse[:], mx[:])
                nc.vector.tensor_mul(stin[:, 2 * E:2 * E + 1], lse[:], lse[:])
                # top-12 of 24 via two max8/match_replace rounds
                work = small.tile([128, E], FP32, tag="work")
                mx8 = small.tile([128, 8], FP32, tag="mx8")
                nc.vector.max(out=mx8[:], in_=pr)
                nc.vector.match_replace(out=work[:], in_to_replace=mx8[:],
                                        in_values=pr, imm_value=0.0)
                mx8b = small.tile([128, 8], FP32, tag="mx8b")
                nc.vector.max(out=mx8b[:], in_=work[:])
                nc.vector.memset(mx8b[:, 4:8], 0.0)
                nc.vector.match_replace(out=work[:], in_to_replace=mx8b[:],
                                        in_values=work[:], imm_value=0.0)
                msk = small.tile([128, E], FP32, tag="msk")
                nc.vector.tensor_sub(msk[:], pr, work[:])
                # gates = masked / (sum(masked) + 1e-6)
                gs = small.tile([128, 1], FP32, tag="gs")
                nc.vector.reduce_sum(out=gs[:], in_=msk[:], axis=AXX)
                nc.vector.tensor_scalar_add(gs[:], gs[:], 1e-6)
                rg = small.tile([128, 1], FP32, tag="rg")
                nc.vector.reciprocal(rg[:], gs[:])
                gm = small.tile([128, E], FP32, tag="gm")
                nc.vector.tensor_scalar_mul(gm[:], msk[:], rg[:])
                nc.vector.tensor_scalar(out=stin[:, E:2 * E], in0=msk[:],
                                        scalar1=1e30, scalar2=1.0,
                                        op0=ALU.mult, op1=ALU.min)
                # loss partials via ones-vector partition reduction on PE
                nc.tensor.matmul(stats[:], ones1[:], stin[:],
                                 start=(m == 0), stop=(m == MT - 1))
                # gates transposed to [E, TQ] for the attention epilogue
                nc.tensor.transpose(gtp[:, m * 128:(m + 1) * 128], gm[:], ident[:])

            nc.vector.tensor_copy(out=gT[:], in_=gtp[:])
            nc.vector.tensor_copy(out=gTbf[:], in_=gtp[:])
            stsb = small.tile([1, 2 * E + 1], FP32, tag="stsb")
            nc.vector.tensor_copy(out=stsb[:], in_=stats[:])
            nc.sync.dma_start(out=p_sum_o[:], in_=stsb[:, 0:E])
            nc.sync.dma_start(out=freqs_o[:], in_=stsb[:, E:2 * E])
            nc.sync.dma_start(out=zsq_o[:], in_=stsb[:, 2 * E:2 * E + 1])

            # per-expert q, pairs packed on out partitions: QT2[p] [128, TQ]
            for p in range(NPAIR):
                qp = ps1b.tile([128, TQ], FP32, tag="qt2p")
                for kc in range(KC):
                    nc.tensor.matmul(
                        qp[:],
                        winbf[:, (p * KC + kc) * 128:(p * KC + kc + 1) * 128],
                        xqbf[:, kc * TQ:(kc + 1) * TQ],
                        start=(kc == 0), stop=(kc == KC - 1))
                nc.vector.tensor_scalar_add(qt2[:, p * TQ:(p + 1) * TQ], qp[:],
                                            binp[:, p:p + 1])

        # ---- stage 2: attention per expert ----
        with tc.tile_pool(name="ps_s", bufs=1, space="PSUM") as ps_s, \
             tc.tile_pool(name="ps_o", bufs=2, space="PSUM") as ps_o, \
             tc.tile_pool(name="ps_a", bufs=2, space="PSUM") as ps_a:
            for e in range(E):
                p, h = divmod(e, 2)
                qrhs = qt2[h * 64:(h + 1) * 64, p * TQ:(p + 1) * TQ]
                ops = ps_o.tile([128, TQ], FP32, tag="ops")
                for rnd in range(NKC // 4):
                    sp = ps_s.tile([128, 4 * 512], FP32, tag="sp")
                    for j in range(4):
                        kk = rnd * 4 + j
                        nc.tensor.matmul(
                            sp[:, j * 512:(j + 1) * 512],
                            ktbf[h * 64:(h + 1) * 64, kk * 128:(kk + 1) * 128],
                            qrhs, start=True, stop=True)
                    pt = pt_pool.tile([128, 4 * 512], BF16, tag="pt")
                    nc.scalar.activation(pt[:], sp[:], AFT.Exp)
                    for j in range(4):
                        kk = rnd * 4 + j
                        nc.tensor.matmul(
                            ops[0:65, :], vauge[:, kk * 65:(kk + 1) * 65],
                            pt[:, j * 512:(j + 1) * 512],
                            start=(kk == 0), stop=(kk == NKC - 1))
                # alpha[t] = gate[e, t] / denom[t], computed entirely at the
                # denominator row's partition (64) since engine ops cannot
                # shift partitions; DMA re-homes the gate row there.
                den = ops[64:65, :]
                grow = rda_pool.tile([128, TQ], FP32, tag="grow")
                nc.sync.dma_start(out=grow[64:65, :], in_=gT[e:e + 1, :])
                rd = rda_pool.tile([128, TQ], FP32, tag="rd")
                nc.vector.reciprocal(rd[64:65, :], den)
                arow = rda_pool.tile([128, TQ], BF16, tag="arow")
                nc.vector.tensor_mul(arow[64:65, :], grow[64:65, :],
                                     rd[64:65, :])
                ap64 = ps_a.tile([64, TQ], FP32, tag="ap64")
                nc.tensor.matmul(ap64[:], ones64[64:65, :], arow[64:65, :],
                                 start=True, stop=True)
                asb = a64_pool.tile([64, TQ], BF16, tag="asb")
                nc.vector.tensor_copy(out=asb[:], in_=ap64[:])
                if h == 0:
                    nc.vector.tensor_mul(
                        ot2[0:64, p * TQ:(p + 1) * TQ], ops[0:64, :], asb[:])
                else:
                    # odd expert: scale at partitions 0:64, then DMA shifts
                    # the result into OT2's upper half (only DMA can move
                    # data across partitions)
                    otmp = a64_pool.tile([64, TQ], BF16, tag="otmp")
                    nc.vector.tensor_mul(otmp[:], ops[0:64, :], asb[:])
                    nc.sync.dma_start(
                        out=ot2[64:128, p * TQ:(p + 1) * TQ], in_=otmp[:])

        # ---- stage 3: gated combine through W_out (pairs packed on K) ----
        with tc.tile_pool(name="ps_y", bufs=2, space="PSUM") as ps_y:
            for m in range(MT):
                yps = []
                for nh in range(2):
                    yp = ps_y.tile([128, 384], FP32, tag=f"yp{nh}")
                    for p in range(NPAIR):
                        nc.tensor.matmul(
                            yp[:],
                            ot2[:, p * TQ + m * 128: p * TQ + (m + 1) * 128],
                            woutbf[:, p * C + nh * 384: p * C + (nh + 1) * 384],
                            start=(p == 0), stop=False)
                    nc.tensor.matmul(
                        yp[:], gTbf[:, m * 128:(m + 1) * 128],
                        boutbf[:, nh * 384:(nh + 1) * 384],
                        start=False, stop=True)
                    yps.append(yp)
                ysb = y_pool.tile([128, C], FP32, tag="ysb")
                nc.vector.tensor_copy(out=ysb[:, 0:384], in_=yps[0][:])
                nc.vector.tensor_copy(out=ysb[:, 384:768], in_=yps[1][:])
                nc.sync.dma_start(out=y_o[m * 128:(m + 1) * 128, :], in_=ysb[:])


def build():
    nc = bacc.Bacc("TRN2", target_bir_lowering=False, debug=False,
                   num_devices=NCORES)
    aps = (
        nc.dram_tensor("xT", [C, N], FP32, kind="ExternalInput").ap(),
        nc.dram_tensor("xqT", [C, TQ], FP32, kind="ExternalInput").ap(),
        nc.dram_tensor("w_gate", [C, E], FP32, kind="ExternalInput").ap(),
        nc.dram_tensor("W_in", [E, C, D], FP32, kind="ExternalInput").ap(),
        nc.dram_tensor("b_in_pairs", [128, NPAIR], FP32, kind="ExternalInput").ap(),
        nc.dram_tensor("W_out", [E, D, C], FP32, kind="ExternalInput").ap(),
        nc.dram_tensor("b_out", [E, C], FP32, kind="ExternalInput").ap(),
        nc.dram_tensor("W_kv", [C, 2 * D], FP32, kind="ExternalInput").ap(),
        nc.dram_tensor("b_k", [128, 1], FP32, kind="ExternalInput").ap(),
        nc.dram_tensor("b_v", [1, D], FP32, kind="ExternalInput").ap(),
        nc.dram_tensor("y", [TQ, C], FP32, kind="ExternalOutput").ap(),
        nc.dram_tensor("p_sum", [1, E], FP32, kind="ExternalOutput").ap(),
        nc.dram_tensor("freqs", [1, E], FP32, kind="ExternalOutput").ap(),
        nc.dram_tensor("zsq", [1, 1], FP32, kind="ExternalOutput").ap(),
    )
    with tile.TileContext(nc) as tc:
        _emit(tc, aps)
    nc.compile()
    return nc


_cache = {}


def _get_nc():
    if "nc" not in _cache:
        _cache["nc"] = build()
    return _cache["nc"]


def make_in_maps(x, w_gate, W_in, b_in, W_out, b_out, W_kv, b_kv):
    x = np.ascontiguousarray(np.asarray(x, np.float32))
    w_gate = np.ascontiguousarray(np.asarray(w_gate, np.float32))
    W_in = np.ascontiguousarray(np.asarray(W_in, np.float32))
    b_in = np.ascontiguousarray(np.asarray(b_in, np.float32))
    W_out = np.ascontiguousarray(np.asarray(W_out, np.float32))
    b_out = np.ascontiguousarray(np.asarray(b_out, np.float32))
    W_kv = np.ascontiguousarray(np.asarray(W_kv, np.float32))
    b_kv = np.ascontiguousarray(np.asarray(b_kv, np.float32))

    xf = x.reshape(T, C)
    b_in_pairs = np.ascontiguousarray(b_in.reshape(NPAIR, 128).T)
    b_k = np.ascontiguousarray(np.tile(b_kv[:D], 2).reshape(128, 1))
    b_v = np.ascontiguousarray(b_kv[D:].reshape(1, D))
    xTs = [np.ascontiguousarray(x[b].T) for b in range(B)]
    in_maps = []
    for c in range(NCORES):
        bidx = (c * TQ) // N
        in_maps.append({
            "xT": xTs[bidx],
            "xqT": np.ascontiguousarray(xf[c * TQ:(c + 1) * TQ].T),
            "w_gate": w_gate,
            "W_in": W_in,
            "b_in_pairs": b_in_pairs,
            "W_out": W_out,
            "b_out": b_out,
            "W_kv": W_kv,
            "b_k": b_k,
            "b_v": b_v,
        })
    return in_maps


def combine_results(results):
    out = np.concatenate([r["y"] for r in results], axis=0).reshape(B, N, C)
    ps = np.sum([r["p_sum"][0] for r in results], axis=0, dtype=np.float32)
    fr = np.sum([r["freqs"][0] for r in results], axis=0, dtype=np.float32)
    zs = np.float32(sum(float(r["zsq"][0, 0]) for r in results))
    switch = np.float32(E) * np.float32(
        np.sum((ps / ps.sum()) * (fr / fr.sum()), dtype=np.float32))
    zloss = zs / np.float32(T)
    aux = np.float32(SWITCHLOSS * switch + ZLOSS * zloss)
    return out, aux


def kernel(x, w_gate, W_in, b_in, W_out, b_out, W_kv, b_kv):
    nc = _get_nc()
    in_maps = make_in_maps(x, w_gate, W_in, b_in, W_out, b_out, W_kv, b_kv)
    trace = os.environ.get("KERNEL_TRACE", "0") == "1"
    res = run_bass_kernel_spmd(nc, in_maps, core_ids=list(range(NCORES)),
                               trace=trace)
    _cache["last_results"] = res
    return combine_results(res.results)


# revision 39
# speedup vs baseline: 2.0574x; 2.0574x over previous
"""MoE-attention (nn_MoEAttention) Trainium2 kernel.

Strategy: data-parallel over tokens across 8 NeuronCores. T = B*N = 4096
tokens; each core owns 512 consecutive query tokens (cores 0-3 -> batch 0,
cores 4-7 -> batch 1), so attention needs no cross-core traffic: every core
gets its batch's full [N, C] activations (transposed on host) to build the
shared K/V.

On-device per core (all FLOPs on device):
  - gating: fp32 logits -> softmax -> top-12 mask (max8/match_replace x2)
    -> dense gates g[t, e] (zero for unselected experts)
  - aux-loss partials: p_sum[E], freqs[E], sum(lse^2) via ones-vector
    matmul partition reductions (combined across cores on the host)
  - per-expert q projections (bf16, expert pairs packed on partitions)
  - attention per expert in transposed layout: S^T[keys, tok] = K^T.T@Q^T,
    P = exp(S^T) (no max subtraction needed: |scores| <= ~2), O^T = V_aug^T@P
    where V_aug carries a ones column so the softmax denominator falls out of
    the same matmul; normalization and the gate are folded into one
    per-token scale applied to O^T.
  - gated combine through W_out with expert pairs packed on the contraction
    axis (full 128-deep matmuls), accumulated in PSUM across experts.

The non-selected experts are computed too (dense) but contribute exactly 0
through g[t, e] = 0 - this avoids any on-device gather/scatter.
"""

import os
from contextlib import ExitStack

import numpy as np

import concourse.bass as bass
import concourse.mybir as mybir
import concourse.tile as tile
from concourse import bacc
from concourse.bass_utils import run_bass_kernel_spmd
from concourse.masks import make_identity

FP32 = mybir.dt.float32
BF16 = mybir.dt.bfloat16
ALU = mybir.AluOpType
AFT = mybir.ActivationFunctionType
AXX = mybir.AxisListType.X

B, N, C, E, D = 2, 2048, 768, 24, 64
H = 12                      # top-k experts per token
T = B * N                   # 4096 tokens
NCORES = 8
TQ = T // NCORES            # 512 query tokens per core
KC = C // 128               # 6 contraction chunks over C
NKC = N // 128              # 16 key chunks
MT = TQ // 128              # 4 output m-tiles per core
NPAIR = E // 2              # 12 expert pairs
SCALE = D ** -0.5

SWITCHLOSS = 0.1
ZLOSS = 0.001


def _emit(tc, aps):
    nc = tc.nc
    (xT, xqT, w_gate, W_in, b_in_pairs, W_out, b_out, W_kv, b_k, b_v,
     y_o, p_sum_o, freqs_o, zsq_o) = aps

    with ExitStack() as ctx:
        persist = ctx.enter_context(tc.tile_pool(name="persist", bufs=1))
        small = ctx.enter_context(tc.tile_pool(name="small", bufs=4))
        rda_pool = ctx.enter_context(tc.tile_pool(name="rda", bufs=2))
        pt_pool = ctx.enter_context(tc.tile_pool(name="pt", bufs=3))
        a64_pool = ctx.enter_context(tc.tile_pool(name="a64", bufs=2))
        y_pool = ctx.enter_context(tc.tile_pool(name="ysb", bufs=2))

        # ---- persistent SBUF tensors ----
        xbf = persist.tile([128, KC * N], BF16, tag="xbf")
        xqf = persist.tile([128, KC * TQ], FP32, tag="xqf")
        xqbf = persist.tile([128, KC * TQ], BF16, tag="xqbf")
        winbf = persist.tile([128, NPAIR * KC * 128], BF16, tag="winbf")
        woutbf = persist.tile([128, NPAIR * C], BF16, tag="woutbf")
        wkvbf = persist.tile([128, KC * 128], BF16, tag="wkvbf")
        wk2 = persist.tile([128, KC * 128], BF16, tag="wk2")
        wg = persist.tile([128, KC * E], FP32, tag="wg")
        # K^T duplicated on both partition halves so either Q half (even /
        # odd expert of a packed pair) can contract against it.
        ktbf = persist.tile([128, N], BF16, tag="ktbf")
        vauge = persist.tile([128, NKC * 65], BF16, tag="vauge")
        qt2 = persist.tile([128, NPAIR * TQ], BF16, tag="qt2")
        ot2 = persist.tile([128, NPAIR * TQ], BF16, tag="ot2")
        gT = persist.tile([E, TQ], FP32, tag="gT")
        gTbf = persist.tile([E, TQ], BF16, tag="gTbf")
        boutbf = persist.tile([E, C], BF16, tag="boutbf")
        binp = persist.tile([128, NPAIR], FP32, tag="binp")
        bksb = persist.tile([128, 1], FP32, tag="bksb")
        bvbf = persist.tile([1, D], BF16, tag="bvbf")
        ident = persist.tile([128, 128], FP32, tag="ident")
        ones1 = persist.tile([128, 1], FP32, tag="ones1")
        onesr = persist.tile([1, 128], BF16, tag="onesr")

        make_identity(nc, ident[:])
        nc.vector.memset(ones1[:], 1.0)
        nc.vector.memset(onesr[:], 1.0)

        # ---- DMA loads (gpsimd DMAs cast fp32 -> bf16 in flight) ----
        for kc in range(KC):
            cs = slice(kc * 128, (kc + 1) * 128)
            nc.gpsimd.dma_start(out=xbf[:, kc * N:(kc + 1) * N], in_=xT[cs, :])
            nc.sync.dma_start(out=xqf[:, kc * TQ:(kc + 1) * TQ], in_=xqT[cs, :])
            nc.gpsimd.dma_start(out=xqbf[:, kc * TQ:(kc + 1) * TQ], in_=xqT[cs, :])
            nc.gpsimd.dma_start(out=wkvbf[:, kc * 128:(kc + 1) * 128], in_=W_kv[cs, :])
            nc.gpsimd.dma_start(out=wk2[:, kc * 128:kc * 128 + 64], in_=W_kv[cs, 0:64])
            nc.gpsimd.dma_start(out=wk2[:, kc * 128 + 64:(kc + 1) * 128], in_=W_kv[cs, 0:64])
            nc.sync.dma_start(out=wg[:, kc * E:(kc + 1) * E], in_=w_gate[cs, :])
        # W_in: one strided cast-DMA per expert (out free dims: [kc, 64] with
        # column step 128 inside the expert-pair block)
        win_view = winbf[:].rearrange("p (pair k h) -> p pair k h",
                                      pair=NPAIR, k=KC, h=128)
        for p in range(NPAIR):
            for hh in range(2):
                nc.gpsimd.dma_start(
                    out=win_view[:, p, :, hh * 64:(hh + 1) * 64],
                    in_=W_in[2 * p + hh].rearrange("(k p) d -> p k d", p=128))
            nc.gpsimd.dma_start(out=woutbf[0:64, p * C:(p + 1) * C], in_=W_out[2 * p])
            nc.gpsimd.dma_start(out=woutbf[64:128, p * C:(p + 1) * C], in_=W_out[2 * p + 1])
        nc.sync.dma_start(out=binp[:], in_=b_in_pairs[:])
        nc.sync.dma_start(out=bksb[:], in_=b_k[:])
        nc.gpsimd.dma_start(out=bvbf[:], in_=b_v[:])
        nc.gpsimd.dma_start(out=boutbf[:], in_=b_out[:])

        # ---- stage 1: K/V, gating, per-expert q (pairs packed on M) ----
        with tc.tile_pool(name="ps1", bufs=1, space="PSUM") as ps1, \
             tc.tile_pool(name="ps1b", bufs=2, space="PSUM") as ps1b:

            # K^T [128, N] (D-major, duplicated halves), scale/bias folded in
            for nb in range(N // 512):
                ktp = ps1b.tile([128, 512], FP32, tag="kt")
                for kc in range(KC):
                    nc.tensor.matmul(
                        ktp[:],
                        wk2[:, kc * 128:(kc + 1) * 128],
                        xbf[:, kc * N + nb * 512: kc * N + (nb + 1) * 512],
                        start=(kc == 0), stop=(kc == KC - 1))
                nc.vector.tensor_scalar(
                    out=ktbf[:, nb * 512:(nb + 1) * 512], in0=ktp[:],
                    scalar1=bksb[:], scalar2=SCALE, op0=ALU.add, op1=ALU.mult)

            # V [keys, 64] -> V_aug variants (ones column for the denominator)
            for kk in range(NKC):
                vps = ps1.tile([128, 64], FP32, tag="vps")
                for kc in range(KC):
                    nc.tensor.matmul(
                        vps[:],
                        xbf[:, kc * N + kk * 128: kc * N + (kk + 1) * 128],
                        wkvbf[:, kc * 128 + 64: kc * 128 + 128],
                        start=(kc == 0), stop=False)
                nc.tensor.matmul(vps[:], onesr[:], bvbf[:], start=False, stop=True)
                nc.vector.tensor_copy(out=vauge[:, kk * 65: kk * 65 + 64], in_=vps[:])
                nc.vector.memset(vauge[:, kk * 65 + 64: kk * 65 + 65], 1.0)

            # gating (exact fp32), top-12 mask, gates, loss partials
            stats = ps1.tile([1, 2 * E + MT], FP32, tag="stats")
            gtp = ps1.tile([E, TQ], FP32, tag="gtp")
            se_all = persist.tile([128, MT], FP32, tag="se_all")
            mx_all = persist.tile([128, MT], FP32, tag="mx_all")
            for m in range(MT):
                lg = ps1.tile([128, E], FP32, tag="lg")
                for kc in range(KC):
                    nc.tensor.matmul(
                        lg[:],
                        xqf[:, kc * TQ + m * 128: kc * TQ + (m + 1) * 128],
                        wg[:, kc * E:(kc + 1) * E],
                        start=(kc == 0), stop=(kc == KC - 1))
                mx = mx_all[:, m:m + 1]
                nc.vector.reduce_max(out=mx, in_=lg[:], axis=AXX)
                negmx = small.tile([128, 1], FP32, tag="negmx")
                nc.vector.tensor_scalar_mul(negmx[:], mx, -1.0)
                se = se_all[:, m:m + 1]
                # stin packs [probs | mask01] so the loss partials ride one
                # PE accumulation group.
                stin = small.tile([128, 2 * E], FP32, tag="stin")
                pr = stin[:, 0:E]
                nc.scalar.activation(pr, lg[:], AFT.Exp, bias=negmx[:], scale=1.0,
                                     accum_out=se)
                rs = small.tile([128, 1], FP32, tag="rs")
                nc.vector.reciprocal(rs[:], se)
                nc.vector.tensor_scalar_mul(pr, pr, rs[:])
                # top-12 of 24 via two max8/match_replace rounds
                work = small.tile([128, E], FP32, tag="work")
                mx8 = small.tile([128, 8], FP32, tag="mx8")
                nc.vector.max(out=mx8[:], in_=pr)
                nc.vector.match_replace(out=work[:], in_to_replace=mx8[:],
                                        in_values=pr, imm_value=0.0)
                mx8b = small.tile([128, 8], FP32, tag="mx8b")
                nc.vector.max(out=mx8b[:], in_=work[:])
                nc.vector.memset(mx8b[:, 4:8], 0.0)
                nc.vector.match_replace(out=work[:], in_to_replace=mx8b[:],
                                        in_values=work[:], imm_value=0.0)
                msk = small.tile([128, E], FP32, tag="msk")
                nc.vector.tensor_sub(msk[:], pr, work[:])
                # gates = masked / (sum(masked) + 1e-6)
                gs = small.tile([128, 1], FP32, tag="gs")
                nc.vector.reduce_sum(out=gs[:], in_=msk[:], axis=AXX)
                nc.vector.tensor_scalar_add(gs[:], gs[:], 1e-6)
                rg = small.tile([128, 1], FP32, tag="rg")
                nc.vector.reciprocal(rg[:], gs[:])
                gm = small.tile([128, E], FP32, tag="gm")
                nc.vector.tensor_scalar_mul(gm[:], msk[:], rg[:])
                nc.vector.tensor_scalar(out=stin[:, E:2 * E], in0=msk[:],
                                        scalar1=1e30, scalar2=1.0,
                                        op0=ALU.mult, op1=ALU.min)
                # loss partials via ones-vector partition reduction on PE
                nc.tensor.matmul(stats[:, 0:2 * E], ones1[:], stin[:],
                                 start=(m == 0), stop=(m == MT - 1))
                # gates transposed to [E, TQ] for the attention epilogue
                nc.tensor.transpose(gtp[:, m * 128:(m + 1) * 128], gm[:], ident[:])

            # z-loss partial: one batched Ln over all 4 m-tiles (avoids
            # alternating Exp/Ln ACT table loads), then (ln(se)+mx)^2
            lnse = small.tile([128, MT], FP32, tag="lnse")
            nc.scalar.activation(lnse[:], se_all[:], AFT.Ln)
            lse = small.tile([128, MT], FP32, tag="lse")
            nc.vector.tensor_add(lse[:], lnse[:], mx_all[:])
            zsq_t = small.tile([128, MT], FP32, tag="zsq_t")
            nc.vector.tensor_mul(zsq_t[:], lse[:], lse[:])
            nc.tensor.matmul(stats[:, 2 * E:2 * E + MT], ones1[:], zsq_t[:],
                             start=True, stop=True)

            nc.vector.tensor_copy(out=gT[:], in_=gtp[:])
            nc.vector.tensor_copy(out=gTbf[:], in_=gtp[:])
            stsb = small.tile([1, 2 * E + MT + 1], FP32, tag="stsb")
            nc.vector.tensor_copy(out=stsb[:, 0:2 * E + MT],
                                  in_=stats[:])
            nc.vector.reduce_sum(out=stsb[:, 2 * E + MT:2 * E + MT + 1],
                                 in_=stsb[:, 2 * E:2 * E + MT], axis=AXX)
            nc.sync.dma_start(out=p_sum_o[:], in_=stsb[:, 0:E])
            nc.sync.dma_start(out=freqs_o[:], in_=stsb[:, E:2 * E])
            nc.sync.dma_start(out=zsq_o[:], in_=stsb[:, 2 * E + MT:2 * E + MT + 1])

            # per-expert q, pairs packed on out partitions: QT2[p] [128, TQ]
            for p in range(NPAIR):
                qp = ps1b.tile([128, TQ], FP32, tag="qt2p")
                for kc in range(KC):
                    nc.tensor.matmul(
                        qp[:],
                        winbf[:, (p * KC + kc) * 128:(p * KC + kc + 1) * 128],
                        xqbf[:, kc * TQ:(kc + 1) * TQ],
                        start=(kc == 0), stop=(kc == KC - 1))
                nc.vector.tensor_scalar_add(qt2[:, p * TQ:(p + 1) * TQ], qp[:],
                                            binp[:, p:p + 1])

        # ---- stage 2: attention per expert ----
        # scores PSUM is double-buffered (2x 3-bank tiles) so the next
        # round's matmuls overlap the current round's exp on ScalarE --
        # without this the PE idles >3.4us per round and the HAM clock gate
        # keeps it at 1.2 GHz for the whole kernel.
        RCH = 3                                 # key chunks per round
        rounds = [(r * RCH, min(RCH, NKC - r * RCH))
                  for r in range((NKC + RCH - 1) // RCH)]
        with tc.tile_pool(name="ps_s", bufs=2, space="PSUM") as ps_s, \
             tc.tile_pool(name="ps_o", bufs=2, space="PSUM") as ps_o:
            for e in range(E):
                p, h = divmod(e, 2)
                qrhs = qt2[h * 64:(h + 1) * 64, p * TQ:(p + 1) * TQ]
                ops = ps_o.tile([128, TQ], FP32, tag="ops")
                for base, cnt in rounds:
                    sp = ps_s.tile([128, RCH * 512], FP32, tag="sp")
                    for j in range(cnt):
                        kk = base + j
                        nc.tensor.matmul(
                            sp[:, j * 512:(j + 1) * 512],
                            ktbf[h * 64:(h + 1) * 64, kk * 128:(kk + 1) * 128],
                            qrhs, start=True, stop=True)
                    pt = pt_pool.tile([128, RCH * 512], BF16, tag="pt")
                    nc.scalar.activation(pt[:, 0:cnt * 512], sp[:, 0:cnt * 512],
                                         AFT.Exp)
                    for j in range(cnt):
                        kk = base + j
                        nc.tensor.matmul(
                            ops[0:65, :], vauge[:, kk * 65:(kk + 1) * 65],
                            pt[:, j * 512:(j + 1) * 512],
                            start=(kk == 0), stop=(kk == NKC - 1))
                # alpha[t] = gate[e, t] / denom[t]. The whole chain runs at
                # partition 0: reciprocal_approx_fast and partition_broadcast
                # both mis-execute at non-zero partition bases on HW, so copy
                # the PSUM denominator row out (DVE, same partition) and DMA
                # it down to partition 0 first.
                dsb = rda_pool.tile([128, TQ], FP32, tag="dsb")
                nc.vector.tensor_copy(out=dsb[64:65, :], in_=ops[64:65, :])
                den0 = rda_pool.tile([1, TQ], FP32, tag="den0")
                nc.sync.dma_start(out=den0[:], in_=dsb[64:65, :])
                grow0 = rda_pool.tile([1, TQ], FP32, tag="grow0")
                nc.sync.dma_start(out=grow0[:], in_=gT[e:e + 1, :])
                rd0 = rda_pool.tile([1, TQ], FP32, tag="rd0")
                nc.vector.reciprocal_approx_fast(out=rd0[:], in_=den0[:])
                arow0 = rda_pool.tile([1, TQ], FP32, tag="arow0")
                nc.vector.tensor_mul(arow0[:], grow0[:], rd0[:])
                # broadcast alpha to 64 partitions on GpSimd (no PSUM, no PE)
                asb = a64_pool.tile([64, TQ], FP32, tag="asb")
                nc.gpsimd.partition_broadcast(asb[:], arow0[:])
                if h == 0:
                    nc.vector.tensor_mul(
                        ot2[0:64, p * TQ:(p + 1) * TQ], ops[0:64, :], asb[:])
                else:
                    # odd expert: scale at partitions 0:64, then DMA shifts
                    # the result into OT2's upper half (only DMA can move
                    # data across partitions)
                    otmp = a64_pool.tile([64, TQ], BF16, tag="otmp")
                    nc.vector.tensor_mul(otmp[:], ops[0:64, :], asb[:])
                    nc.sync.dma_start(
                        out=ot2[64:128, p * TQ:(p + 1) * TQ], in_=otmp[:])

        # ---- stage 3: gated combine through W_out (pairs packed on K) ----
        with tc.tile_pool(name="ps_y", bufs=2, space="PSUM") as ps_y:
            for m in range(MT):
                yps = []
                for nh in range(2):
                    yp = ps_y.tile([128, 384], FP32, tag=f"yp{nh}")
                    for p in range(NPAIR):
                        nc.tensor.matmul(
                            yp[:],
                            ot2[:, p * TQ + m * 128: p * TQ + (m + 1) * 128],
                            woutbf[:, p * C + nh * 384: p * C + (nh + 1) * 384],
                            start=(p == 0), stop=False)
                    nc.tensor.matmul(
                        yp[:], gTbf[:, m * 128:(m + 1) * 128],
                        boutbf[:, nh * 384:(nh + 1) * 384],
                        start=False, stop=True)
                    yps.append(yp)
                ysb = y_pool.tile([128, C], FP32, tag="ysb")
                nc.vector.tensor_copy(out=ysb[:, 0:384], in_=yps[0][:])
                nc.vector.tensor_copy(out=ysb[:, 384:768], in_=yps[1][:])
                nc.sync.dma_start(out=y_o[m * 128:(m + 1) * 128, :], in_=ysb[:])


def build():
    nc = bacc.Bacc("TRN2", target_bir_lowering=False, debug=False,
                   num_devices=NCORES)
    aps = (
        nc.dram_tensor("xT", [C, N], FP32, kind="ExternalInput").ap(),
        nc.dram_tensor("xqT", [C, TQ], FP32, kind="ExternalInput").ap(),
        nc.dram_tensor("w_gate", [C, E], FP32, kind="ExternalInput").ap(),
        nc.dram_tensor("W_in", [E, C, D], FP32, kind="ExternalInput").ap(),
        nc.dram_tensor("b_in_pairs", [128, NPAIR], FP32, kind="ExternalInput").ap(),
        nc.dram_tensor("W_out", [E, D, C], FP32, kind="ExternalInput").ap(),
        nc.dram_tensor("b_out", [E, C], FP32, kind="ExternalInput").ap(),
        nc.dram_tensor("W_kv", [C, 2 * D], FP32, kind="ExternalInput").ap(),
        nc.dram_tensor("b_k", [128, 1], FP32, kind="ExternalInput").ap(),
        nc.dram_tensor("b_v", [1, D], FP32, kind="ExternalInput").ap(),
        nc.dram_tensor("y", [TQ, C], FP32, kind="ExternalOutput").ap(),
        nc.dram_tensor("p_sum", [1, E], FP32, kind="ExternalOutput").ap(),
        nc.dram_tensor("freqs", [1, E], FP32, kind="ExternalOutput").ap(),
        nc.dram_tensor("zsq", [1, 1], FP32, kind="ExternalOutput").ap(),
    )
    with tile.TileContext(nc) as tc:
        _emit(tc, aps)
    nc.compile()
    return nc


_cache = {}


def _get_nc():
    if "nc" not in _cache:
        _cache["nc"] = build()
    return _cache["nc"]


def make_in_maps(x, w_gate, W_in, b_in, W_out, b_out, W_kv, b_kv):
    x = np.ascontiguousarray(np.asarray(x, np.float32))
    w_gate = np.ascontiguousarray(np.asarray(w_gate, np.float32))
    W_in = np.ascontiguousarray(np.asarray(W_in, np.float32))
    b_in = np.ascontiguousarray(np.asarray(b_in, np.float32))
    W_out = np.ascontiguousarray(np.asarray(W_out, np.float32))
    b_out = np.ascontiguousarray(np.asarray(b_out, np.float32))
    W_kv = np.ascontiguousarray(np.asarray(W_kv, np.float32))
    b_kv = np.ascontiguousarray(np.asarray(b_kv, np.float32))

    xf = x.reshape(T, C)
    b_in_pairs = np.ascontiguousarray(b_in.reshape(NPAIR, 128).T)
    b_k = np.ascontiguousarray(np.tile(b_kv[:D], 2).reshape(128, 1))
    b_v = np.ascontiguousarray(b_kv[D:].reshape(1, D))
    xTs = [np.ascontiguousarray(x[b].T) for b in range(B)]
    in_maps = []
    for c in range(NCORES):
        bidx = (c * TQ) // N
        in_maps.append({
            "xT": xTs[bidx],
            "xqT": np.ascontiguousarray(xf[c * TQ:(c + 1) * TQ].T),
            "w_gate": w_gate,
            "W_in": W_in,
            "b_in_pairs": b_in_pairs,
            "W_out": W_out,
            "b_out": b_out,
            "W_kv": W_kv,
            "b_k": b_k,
            "b_v": b_v,
        })
    return in_maps


def combine_results(results):
    out = np.concatenate([r["y"] for r in results], axis=0).reshape(B, N, C)
    ps = np.sum([r["p_sum"][0] for r in results], axis=0, dtype=np.float32)
    fr = np.sum([r["freqs"][0] for r in results], axis=0, dtype=np.float32)
    zs = np.float32(sum(float(r["zsq"][0, 0]) for r in results))
    switch = np.float32(E) * np.float32(
        np.sum((ps / ps.sum()) * (fr / fr.sum()), dtype=np.float32))
    zloss = zs / np.float32(T)
    aux = np.float32(SWITCHLOSS * switch + ZLOSS * zloss)
    return out, aux


def kernel(x, w_gate, W_in, b_in, W_out, b_out, W_kv, b_kv):
    nc = _get_nc()
    in_maps = make_in_maps(x, w_gate, W_in, b_in, W_out, b_out, W_kv, b_kv)
    trace = os.environ.get("KERNEL_TRACE", "0") == "1"
    res = run_bass_kernel_spmd(nc, in_maps, core_ids=list(range(NCORES)),
                               trace=trace)
    _cache["last_results"] = res
    return combine_results(res.results)
